# revision 26
# baseline (speedup 1.0000x reference)
"""EDAC layer kernel for Trainium2 (8 NeuronCores, batch-sharded SPMD).

Reference semantics (B=32, C=256, K=64, H=W=56; vulnerable_idx == arange(K)):
  valid(x, c)  = min_vals[c] <= x <= max_vals[c]
  channels >= K:  out = x if valid else 0
  channels <  K:  m = main, d = dup
      both valid  -> min(m, d)      (covers m == d too)
      only d      -> d
      only m      -> m
      neither     -> 0

Kernel strategy (per core, 4 batches):
  rows = (batch, channel) pairs on SBUF partitions, H*W on the free dim.
  Per batch-pair (b, b+1) process five [128, HW] tiles:
    A: batch b   channels  64..191   (simple range-zero path)
    B: batch b   channels 192..255 + batch b+1 channels 64..127
    C: batch b+1 channels 128..255
    V: channels 0..63 of both batches (vulnerable, compared against dup)
    D: dup rows for both batches
  Simple path: two in-place scalar_tensor_tensor ops on VectorE
               ((m>=lo)*m, then (m<=hi)*that -- safe because 0 <= hi).
  Vulnerable:  ScalarE relus r1=relu(lo-x), r2=relu(x-hi) in bf16 (zero vs
               positive is exact), w = r1+r2 via TensorE identity-matmul
               accumulation into PSUM, sentinel m1 = m + HUGE*w on VectorE,
               r = min(m1, d1), res = (r < THR) * r.
  Engine/DMA plan: loads on the sync HWDGE ring (single FIFO = lowest
  first-tile latency), early stores via GPSIMD SWDGE, late stores on the
  then-idle sync ring.  B/V/D tiles interleave their two 64-row segments
  into even/odd partitions via [64, 2, hw] APs so every DMA keeps full
  128-partition port coverage across all 16 SDMA engines.
"""

import os
import sys

for _p in ("/opt/trn_rl_repo", os.path.expanduser("~/.axon_site/_ro/trn_rl_repo")):
    if os.path.isdir(_p) and _p not in sys.path:
        sys.path.insert(0, _p)

import numpy as np

import concourse.bass as bass
import concourse.bacc as bacc
import concourse.mybir as mybir
from concourse.tile import TileContext
from concourse.bass_utils import run_bass_kernel_spmd

F32 = mybir.dt.float32
BF16 = mybir.dt.bfloat16
OP = mybir.AluOpType
AF = mybir.ActivationFunctionType

B, C, K, H, W = 32, 256, 64, 56, 56
HW = H * W
NCORES = 8
BL = B // NCORES  # batches per core

HUGE = 1.0e30  # sentinel multiplier: HUGE * smallest-positive-bf16-relu >> THR
THR = 1.0e15   # valid values are <= ~10; invalid sentinels are >= ~6e22

# bounds table columns (per-partition scalars for each tile kind)
#   0..3  : lo  for tile kinds A, B, C, V
#   4..7  : hi  for tile kinds A, B, C, V
#   8..11 : -hi for tile kinds A, B, C, V
NBCOLS = 12


def build_bounds(min_vals: np.ndarray, max_vals: np.ndarray) -> np.ndarray:
    lo = np.asarray(min_vals, dtype=np.float32)
    hi = np.asarray(max_vals, dtype=np.float32)
    cols = np.zeros((128, NBCOLS), dtype=np.float32)
    interleave = lambda a, b: np.stack([a, b], axis=1).ravel()
    kinds = [
        np.arange(64, 192),                                   # A: ch 64..191
        interleave(np.arange(192, 256), np.arange(64, 128)),  # B (interleaved)
        np.arange(128, 256),                                  # C: ch 128..255
        np.repeat(np.arange(0, 64), 2),                       # V (interleaved)
    ]
    for j, idx in enumerate(kinds):
        cols[:, j] = lo[idx]
        cols[:, 4 + j] = hi[idx]
        cols[:, 8 + j] = -hi[idx]
    return cols


def build_nc(hw: int = HW) -> bass.Bass:
    nc = bacc.Bacc("TRN2", target_bir_lowering=False, debug=False)
    R = BL * C
    main = nc.dram_tensor("main", [R, hw], F32, kind="ExternalInput")
    dup = nc.dram_tensor("dup", [BL * K, hw], F32, kind="ExternalInput")
    bounds = nc.dram_tensor("bounds", [128, NBCOLS], F32, kind="ExternalInput")
    ident = nc.dram_tensor("ident", [128, 128], BF16, kind="ExternalInput")
    out = nc.dram_tensor("out", [R, hw], F32, kind="ExternalOutput")

    stt = nc.vector.scalar_tensor_tensor
    npairs = BL // 2

    # Per-pair DRAM views. B and V tiles interleave their two 64-row segments
    # into even/odd SBUF partitions via a [64, 2, hw] AP (outer dim 64), so a
    # single dma_start still spreads over all 16 SDMA engines with full
    # 128-partition port coverage (64-partition DMAs run at half BW; multi-
    # segment outer-dim-2 APs collapse onto 2 engines).
    main_p = main.ap().rearrange("(p x) w -> p x w", p=npairs)   # [p, 512, hw]
    out_p = out.ap().rearrange("(p x) w -> p x w", p=npairs)
    dup_p = dup.ap().rearrange("(p s c) w -> p c s w", p=npairs, s=2)

    def v_ap(t):   # [64, 2, hw]: ch 0..63 of batches b, b+1 interleaved
        return t.rearrange("(s g c) w -> g c s w", s=2, g=4)[0]

    def b_ap(t):   # [64, 2, hw]: ch 192..255 of b / ch 64..127 of b+1
        return t[192:384].rearrange("(s c) w -> c s w", s=3)[:, 0:3:2]

    APS = {
        0: lambda t: t[64:192],      # A
        1: b_ap,                     # B
        2: lambda t: t[384:512],     # C
    }

    with TileContext(nc) as tc:
        with (
            tc.tile_pool(name="bnd", bufs=1) as bpool,
            tc.tile_pool(name="pm", bufs=6) as pm,
            tc.tile_pool(name="pv", bufs=2) as pv,
            tc.tile_pool(name="pd", bufs=2) as pd,
            tc.tile_pool(name="pr", bufs=12) as pr,
            tc.tile_pool(name="pp", bufs=2, space="PSUM") as pp,
        ):
            bt = bpool.tile([128, NBCOLS], F32)
            nc.sync.dma_start(out=bt[:], in_=bounds[:])
            it = bpool.tile([128, 128], BF16, tag="ident")
            nc.sync.dma_start(out=it[:], in_=ident[:])

            def lo_ap(j):
                return bt[:, j:j + 1]

            def hi_ap(j):
                return bt[:, 4 + j:5 + j]

            def nhi_ap(j):
                return bt[:, 8 + j:9 + j]

            # Load-trigger order (scalar HWDGE ring) is tuned so the DVE
            # starts on A0 at ~13us while V/D of each pair still land early
            # enough to hide the ScalarE relu chain behind simple-tile DVE
            # work.  Tiles land ~4.4us apart while the ring streams.
            vd = [None] * npairs
            abc = [[None] * 3 for _ in range(npairs)]

            def load_vd(p):
                mv = pv.tile([128, hw], F32, tag="mv")
                nc.sync.dma_start(out=mv[:], in_=v_ap(main_p[p]))
                dv = pd.tile([128, hw], F32, tag="dv")
                nc.sync.dma_start(out=dv[:], in_=dup_p[p])
                vd[p] = (mv, dv)

            def load_simple(p, kind, split=False):
                mt = pm.tile([128, hw], F32, tag="mt")
                src_ap = APS[kind](main_p[p])
                if split:  # two half DMAs: first data lands sooner
                    h = hw // 2
                    nc.sync.dma_start(out=mt[:, 0:h], in_=src_ap[..., 0:h])
                    nc.sync.dma_start(out=mt[:, h:hw], in_=src_ap[..., h:hw])
                else:
                    nc.sync.dma_start(out=mt[:], in_=src_ap)
                abc[p][kind] = mt

            load_simple(0, 0, split=True)
            load_vd(0)
            load_simple(0, 1)
            load_vd(1)
            load_simple(0, 2)
            load_simple(1, 0)
            load_simple(1, 1)
            load_simple(1, 2)

            # ScalarE relu stream: vuln pairs first, then the two simple
            # tiles that take the relu+PE path (A1, B1).
            relus = []
            for p in range(npairs):
                mv, dv = vd[p]
                r1m = pr.tile([128, hw], BF16, tag="rl")
                r2m = pr.tile([128, hw], BF16, tag="rl")
                r1d = pr.tile([128, hw], BF16, tag="rl")
                r2d = pr.tile([128, hw], BF16, tag="rl")
                nc.scalar.activation(r1m[:], mv[:], AF.Relu, bias=lo_ap(3), scale=-1.0)
                nc.scalar.activation(r2m[:], mv[:], AF.Relu, bias=nhi_ap(3), scale=1.0)
                nc.scalar.activation(r1d[:], dv[:], AF.Relu, bias=lo_ap(3), scale=-1.0)
                nc.scalar.activation(r2d[:], dv[:], AF.Relu, bias=nhi_ap(3), scale=1.0)
                relus.append((r1m, r2m, r1d, r2d))
            # relu+PE path for the two latest simple tiles: their data lands
            # early enough and ScalarE is idle once the vuln relus finish,
            # while their DVE slots come ~15us later.
            srelus = {}
            for p, kind in ((1, 1), (1, 2)):
                mt = abc[p][kind]
                r1 = pr.tile([128, hw], BF16, tag="rl")
                r2 = pr.tile([128, hw], BF16, tag="rl")
                nc.scalar.activation(r1[:], mt[:], AF.Relu, bias=lo_ap(kind),
                                     scale=-1.0)
                nc.scalar.activation(r2[:], mt[:], AF.Relu, bias=nhi_ap(kind),
                                     scale=1.0)
                srelus[(p, kind)] = (r1, r2)
            half = hw // 2

            def pe_w(r1, r2, cs):
                """w = r1 + r2 on TensorE (identity matmuls into PSUM)."""
                w = pp.tile([128, half], F32, tag="w")
                for c0 in range(0, half, 512):
                    c1 = min(c0 + 512, half)
                    nc.tensor.matmul(w[:, c0:c1], it[:], r1[:, cs][:, c0:c1],
                                     start=True, stop=False)
                    nc.tensor.matmul(w[:, c0:c1], it[:], r2[:, cs][:, c0:c1],
                                     start=False, stop=True)
                return w

            def do_simple(p, kind, late=False, split=False):
                mt = abc[p][kind]
                eng = nc.sync if late else nc.gpsimd
                dst = APS[kind](out_p[p])
                halves = ((slice(0, half), slice(half, hw))
                          if split else (slice(0, hw),))
                for cs in halves:
                    stt(out=mt[:, cs], in0=mt[:, cs], scalar=lo_ap(kind),
                        in1=mt[:, cs], op0=OP.is_ge, op1=OP.mult)
                    stt(out=mt[:, cs], in0=mt[:, cs], scalar=hi_ap(kind),
                        in1=mt[:, cs], op0=OP.is_le, op1=OP.mult)
                    eng.dma_start(out=dst[..., cs], in_=mt[:, cs])

            def do_simple_relu(p, kind, late=False):
                # out = (w == 0) * m: one stt per half against the PSUM w
                mt = abc[p][kind]
                r1, r2 = srelus[(p, kind)]
                eng = nc.sync if late else nc.gpsimd
                dst = APS[kind](out_p[p])
                for h in range(2):
                    cs = slice(h * half, (h + 1) * half)
                    w = pe_w(r1, r2, cs)
                    stt(out=mt[:, cs], in0=w[:], scalar=0.0, in1=mt[:, cs],
                        op0=OP.is_equal, op1=OP.mult)
                    eng.dma_start(out=dst[..., cs], in_=mt[:, cs])

            def do_vuln(p):
                mv, dv = vd[p]
                r1m, r2m, r1d, r2d = relus[p]
                for src_pair, dst in (((r1m, r2m), mv), ((r1d, r2d), dv)):
                    for h in range(2):
                        cs = slice(h * half, (h + 1) * half)
                        w = pe_w(src_pair[0], src_pair[1], cs)
                        stt(out=dst[:, cs], in0=w[:], scalar=HUGE,
                            in1=dst[:, cs], op0=OP.mult, op1=OP.add)
                eng = nc.sync if p == npairs - 1 else nc.gpsimd
                vdst = v_ap(out_p[p])
                for h in range(2):
                    cs = slice(h * half, (h + 1) * half)
                    nc.vector.tensor_tensor(out=mv[:, cs], in0=mv[:, cs],
                                            in1=dv[:, cs], op=OP.min)
                    stt(out=dv[:, cs], in0=mv[:, cs], scalar=THR,
                        in1=mv[:, cs], op0=OP.is_lt, op1=OP.mult)
                    eng.dma_start(out=vdst[..., cs], in_=dv[:, cs])

            do_simple(0, 0, split=True)
            do_simple(0, 1)
            do_vuln(0)
            do_simple(0, 2)
            do_simple(1, 0)
            do_vuln(1)
            do_simple_relu(1, 1, late=True)
            do_simple_relu(1, 2, late=True)
    return nc


_NC_CACHE: dict = {}


def _get_nc(hw: int) -> bass.Bass:
    if hw not in _NC_CACHE:
        nc = build_nc(hw)
        nc.finalize()  # Bacc.finalize runs compile() (register allocation etc.)
        _NC_CACHE[hw] = nc
    return _NC_CACHE[hw]


def kernel(main_out, dup_out, min_vals, max_vals, vulnerable_idx):
    return _run(main_out, dup_out, min_vals, max_vals, vulnerable_idx)[0]


def _run(main_out, dup_out, min_vals, max_vals, vulnerable_idx, **spmd_kwargs):
    main_out = np.asarray(main_out)
    dup_out = np.asarray(dup_out)
    min_vals = np.asarray(min_vals)
    max_vals = np.asarray(max_vals)
    vidx = np.asarray(vulnerable_idx).ravel()

    # Device kernel assumes vulnerable channels are 0..K-1. If not, permute
    # channels host-side so they are, and invert on the way out.
    perm = None
    if not np.array_equal(vidx, np.arange(K)):
        assert len(np.unique(vidx)) == K, "duplicate vulnerable_idx unsupported"
        rest = np.setdiff1d(np.arange(C), vidx)
        perm = np.concatenate([vidx, rest])
        main_out = main_out[:, perm]
        min_vals = min_vals[perm]
        max_vals = max_vals[perm]

    mo = np.ascontiguousarray(main_out, dtype=np.float32).reshape(B, C, HW)
    du = np.ascontiguousarray(dup_out, dtype=np.float32).reshape(B, K, HW)
    bounds = build_bounds(min_vals, max_vals)
    import ml_dtypes
    ident = np.eye(128, dtype=ml_dtypes.bfloat16)

    in_maps = []
    for k in range(NCORES):
        in_maps.append({
            "main": mo[BL * k:BL * (k + 1)].reshape(BL * C, HW),
            "dup": du[BL * k:BL * (k + 1)].reshape(BL * K, HW),
            "bounds": bounds,
            "ident": ident,
        })

    nc = _get_nc(HW)
    res = run_bass_kernel_spmd(nc, in_maps, list(range(NCORES)), **spmd_kwargs)
    out = np.concatenate(
        [r["out"].reshape(BL, C, H, W) for r in res.results], axis=0)

    if perm is not None:
        inv = np.empty(C, dtype=np.int64)
        inv[perm] = np.arange(C)
        out = out[:, inv]
    return out, res


# revision 27
# speedup vs baseline: 1.0059x; 1.0059x over previous
"""EDAC layer kernel for Trainium2 (8 NeuronCores, batch-sharded SPMD).

Reference semantics (B=32, C=256, K=64, H=W=56; vulnerable_idx == arange(K)):
  valid(x, c)  = min_vals[c] <= x <= max_vals[c]
  channels >= K:  out = x if valid else 0
  channels <  K:  m = main, d = dup
      both valid  -> min(m, d)      (covers m == d too)
      only d      -> d
      only m      -> m
      neither     -> 0

Kernel strategy (per core, 4 batches):
  rows = (batch, channel) pairs on SBUF partitions, H*W on the free dim.
  Per batch-pair (b, b+1) process five [128, HW] tiles:
    A: batch b   channels  64..191   (simple range-zero path)
    B: batch b   channels 192..255 + batch b+1 channels 64..127
    C: batch b+1 channels 128..255
    V: channels 0..63 of both batches (vulnerable, compared against dup)
    D: dup rows for both batches
  Simple path: two in-place scalar_tensor_tensor ops on VectorE
               ((m>=lo)*m, then (m<=hi)*that -- safe because 0 <= hi).
  Vulnerable:  ScalarE relus r1=relu(lo-x), r2=relu(x-hi) in bf16 (zero vs
               positive is exact), w = r1+r2 via TensorE identity-matmul
               accumulation into PSUM, sentinel m1 = m + HUGE*w on VectorE,
               r = min(m1, d1), res = (r < THR) * r.
  Engine/DMA plan: loads on the sync HWDGE ring (single FIFO = lowest
  first-tile latency), early stores via GPSIMD SWDGE, late stores on the
  then-idle sync ring.  B/V/D tiles interleave their two 64-row segments
  into even/odd partitions via [64, 2, hw] APs so every DMA keeps full
  128-partition port coverage across all 16 SDMA engines.
"""

import os
import sys

for _p in ("/opt/trn_rl_repo", os.path.expanduser("~/.axon_site/_ro/trn_rl_repo")):
    if os.path.isdir(_p) and _p not in sys.path:
        sys.path.insert(0, _p)

import numpy as np

import concourse.bass as bass
import concourse.bacc as bacc
import concourse.mybir as mybir
from concourse.tile import TileContext
from concourse.bass_utils import run_bass_kernel_spmd

F32 = mybir.dt.float32
BF16 = mybir.dt.bfloat16
OP = mybir.AluOpType
AF = mybir.ActivationFunctionType

B, C, K, H, W = 32, 256, 64, 56, 56
HW = H * W
NCORES = 8
BL = B // NCORES  # batches per core

HUGE = 1.0e30  # sentinel multiplier: HUGE * smallest-positive-bf16-relu >> THR
THR = 1.0e15   # valid values are <= ~10; invalid sentinels are >= ~6e22

# bounds table columns (per-partition scalars for each tile kind)
#   0..3  : lo  for tile kinds A, B, C, V
#   4..7  : hi  for tile kinds A, B, C, V
#   8..11 : -hi for tile kinds A, B, C, V
NBCOLS = 12


def build_bounds(min_vals: np.ndarray, max_vals: np.ndarray) -> np.ndarray:
    lo = np.asarray(min_vals, dtype=np.float32)
    hi = np.asarray(max_vals, dtype=np.float32)
    cols = np.zeros((128, NBCOLS), dtype=np.float32)
    interleave = lambda a, b: np.stack([a, b], axis=1).ravel()
    kinds = [
        np.arange(64, 192),                                   # A: ch 64..191
        interleave(np.arange(192, 256), np.arange(64, 128)),  # B (interleaved)
        np.arange(128, 256),                                  # C: ch 128..255
        np.repeat(np.arange(0, 64), 2),                       # V (interleaved)
    ]
    for j, idx in enumerate(kinds):
        cols[:, j] = lo[idx]
        cols[:, 4 + j] = hi[idx]
        cols[:, 8 + j] = -hi[idx]
    return cols


def build_nc(hw: int = HW) -> bass.Bass:
    nc = bacc.Bacc("TRN2", target_bir_lowering=False, debug=False)
    R = BL * C
    main = nc.dram_tensor("main", [R, hw], F32, kind="ExternalInput")
    dup = nc.dram_tensor("dup", [BL * K, hw], F32, kind="ExternalInput")
    bounds = nc.dram_tensor("bounds", [128, NBCOLS], F32, kind="ExternalInput")
    ident = nc.dram_tensor("ident", [128, 128], BF16, kind="ExternalInput")
    out = nc.dram_tensor("out", [R, hw], F32, kind="ExternalOutput")

    stt = nc.vector.scalar_tensor_tensor
    npairs = BL // 2

    # Per-pair DRAM views. B and V tiles interleave their two 64-row segments
    # into even/odd SBUF partitions via a [64, 2, hw] AP (outer dim 64), so a
    # single dma_start still spreads over all 16 SDMA engines with full
    # 128-partition port coverage (64-partition DMAs run at half BW; multi-
    # segment outer-dim-2 APs collapse onto 2 engines).
    main_p = main.ap().rearrange("(p x) w -> p x w", p=npairs)   # [p, 512, hw]
    out_p = out.ap().rearrange("(p x) w -> p x w", p=npairs)
    dup_p = dup.ap().rearrange("(p s c) w -> p c s w", p=npairs, s=2)

    def v_ap(t):   # [64, 2, hw]: ch 0..63 of batches b, b+1 interleaved
        return t.rearrange("(s g c) w -> g c s w", s=2, g=4)[0]

    def b_ap(t):   # [64, 2, hw]: ch 192..255 of b / ch 64..127 of b+1
        return t[192:384].rearrange("(s c) w -> c s w", s=3)[:, 0:3:2]

    APS = {
        0: lambda t: t[64:192],      # A
        1: b_ap,                     # B
        2: lambda t: t[384:512],     # C
    }

    with TileContext(nc) as tc:
        with (
            tc.tile_pool(name="bnd", bufs=1) as bpool,
            tc.tile_pool(name="pm", bufs=6) as pm,
            tc.tile_pool(name="pv", bufs=2) as pv,
            tc.tile_pool(name="pd", bufs=2) as pd,
            tc.tile_pool(name="pr", bufs=8) as pr,
            tc.tile_pool(name="pp", bufs=2, space="PSUM") as pp,
        ):
            bt = bpool.tile([128, NBCOLS], F32)
            nc.sync.dma_start(out=bt[:], in_=bounds[:])
            it = bpool.tile([128, 128], BF16, tag="ident")
            nc.sync.dma_start(out=it[:], in_=ident[:])

            def lo_ap(j):
                return bt[:, j:j + 1]

            def hi_ap(j):
                return bt[:, 4 + j:5 + j]

            def nhi_ap(j):
                return bt[:, 8 + j:9 + j]

            # Load-trigger order (scalar HWDGE ring) is tuned so the DVE
            # starts on A0 at ~13us while V/D of each pair still land early
            # enough to hide the ScalarE relu chain behind simple-tile DVE
            # work.  Tiles land ~4.4us apart while the ring streams.
            vd = [None] * npairs
            abc = [[None] * 3 for _ in range(npairs)]

            def load_vd(p):
                mv = pv.tile([128, hw], F32, tag="mv")
                nc.sync.dma_start(out=mv[:], in_=v_ap(main_p[p]))
                dv = pd.tile([128, hw], F32, tag="dv")
                nc.sync.dma_start(out=dv[:], in_=dup_p[p])
                vd[p] = (mv, dv)

            def load_simple(p, kind, split=False):
                mt = pm.tile([128, hw], F32, tag="mt")
                src_ap = APS[kind](main_p[p])
                if split:  # two half DMAs: first data lands sooner
                    h = hw // 2
                    nc.sync.dma_start(out=mt[:, 0:h], in_=src_ap[..., 0:h])
                    nc.sync.dma_start(out=mt[:, h:hw], in_=src_ap[..., h:hw])
                else:
                    nc.sync.dma_start(out=mt[:], in_=src_ap)
                abc[p][kind] = mt

            load_simple(0, 0, split=True)
            load_vd(0)
            load_simple(0, 1)
            load_vd(1)
            load_simple(0, 2)
            load_simple(1, 0)
            load_simple(1, 1)
            load_simple(1, 2)

            # ScalarE relu stream: vuln pairs first, then the two simple
            # tiles that take the relu+PE path (A1, B1).
            relus = []
            for p in range(npairs):
                mv, dv = vd[p]
                r1m = pr.tile([128, hw], BF16, tag="rl")
                r2m = pr.tile([128, hw], BF16, tag="rl")
                r1d = pr.tile([128, hw], BF16, tag="rl")
                r2d = pr.tile([128, hw], BF16, tag="rl")
                nc.scalar.activation(r1m[:], mv[:], AF.Relu, bias=lo_ap(3), scale=-1.0)
                nc.scalar.activation(r2m[:], mv[:], AF.Relu, bias=nhi_ap(3), scale=1.0)
                nc.scalar.activation(r1d[:], dv[:], AF.Relu, bias=lo_ap(3), scale=-1.0)
                nc.scalar.activation(r2d[:], dv[:], AF.Relu, bias=nhi_ap(3), scale=1.0)
                relus.append((r1m, r2m, r1d, r2d))
            half = hw // 2

            def pe_w(r1, r2, cs):
                """w = r1 + r2 on TensorE (identity matmuls into PSUM)."""
                w = pp.tile([128, half], F32, tag="w")
                for c0 in range(0, half, 512):
                    c1 = min(c0 + 512, half)
                    nc.tensor.matmul(w[:, c0:c1], it[:], r1[:, cs][:, c0:c1],
                                     start=True, stop=False)
                    nc.tensor.matmul(w[:, c0:c1], it[:], r2[:, cs][:, c0:c1],
                                     start=False, stop=True)
                return w

            def do_simple(p, kind, late=False, split=False):
                mt = abc[p][kind]
                eng = nc.sync
                dst = APS[kind](out_p[p])
                halves = ((slice(0, half), slice(half, hw))
                          if split else (slice(0, hw),))
                for cs in halves:
                    stt(out=mt[:, cs], in0=mt[:, cs], scalar=lo_ap(kind),
                        in1=mt[:, cs], op0=OP.is_ge, op1=OP.mult)
                    stt(out=mt[:, cs], in0=mt[:, cs], scalar=hi_ap(kind),
                        in1=mt[:, cs], op0=OP.is_le, op1=OP.mult)
                    eng.dma_start(out=dst[..., cs], in_=mt[:, cs])

            def do_vuln(p):
                mv, dv = vd[p]
                r1m, r2m, r1d, r2d = relus[p]
                for src_pair, dst in (((r1m, r2m), mv), ((r1d, r2d), dv)):
                    for h in range(2):
                        cs = slice(h * half, (h + 1) * half)
                        w = pe_w(src_pair[0], src_pair[1], cs)
                        stt(out=dst[:, cs], in0=w[:], scalar=HUGE,
                            in1=dst[:, cs], op0=OP.mult, op1=OP.add)
                eng = nc.sync
                vdst = v_ap(out_p[p])
                for h in range(2):
                    cs = slice(h * half, (h + 1) * half)
                    nc.vector.tensor_tensor(out=mv[:, cs], in0=mv[:, cs],
                                            in1=dv[:, cs], op=OP.min)
                    stt(out=dv[:, cs], in0=mv[:, cs], scalar=THR,
                        in1=mv[:, cs], op0=OP.is_lt, op1=OP.mult)
                    eng.dma_start(out=vdst[..., cs], in_=dv[:, cs])

            do_simple(0, 0, split=True)
            do_simple(0, 1)
            do_vuln(0)
            do_simple(0, 2)
            do_simple(1, 0)
            do_vuln(1)
            do_simple(1, 1, late=True)
            do_simple(1, 2, late=True, split=True)
    return nc


_NC_CACHE: dict = {}


def _get_nc(hw: int) -> bass.Bass:
    if hw not in _NC_CACHE:
        nc = build_nc(hw)
        nc.finalize()  # Bacc.finalize runs compile() (register allocation etc.)
        _NC_CACHE[hw] = nc
    return _NC_CACHE[hw]


def kernel(main_out, dup_out, min_vals, max_vals, vulnerable_idx):
    return _run(main_out, dup_out, min_vals, max_vals, vulnerable_idx)[0]


def _run(main_out, dup_out, min_vals, max_vals, vulnerable_idx, **spmd_kwargs):
    main_out = np.asarray(main_out)
    dup_out = np.asarray(dup_out)
    min_vals = np.asarray(min_vals)
    max_vals = np.asarray(max_vals)
    vidx = np.asarray(vulnerable_idx).ravel()

    # Device kernel assumes vulnerable channels are 0..K-1. If not, permute
    # channels host-side so they are, and invert on the way out.
    perm = None
    if not np.array_equal(vidx, np.arange(K)):
        assert len(np.unique(vidx)) == K, "duplicate vulnerable_idx unsupported"
        rest = np.setdiff1d(np.arange(C), vidx)
        perm = np.concatenate([vidx, rest])
        main_out = main_out[:, perm]
        min_vals = min_vals[perm]
        max_vals = max_vals[perm]

    mo = np.ascontiguousarray(main_out, dtype=np.float32).reshape(B, C, HW)
    du = np.ascontiguousarray(dup_out, dtype=np.float32).reshape(B, K, HW)
    bounds = build_bounds(min_vals, max_vals)
    import ml_dtypes
    ident = np.eye(128, dtype=ml_dtypes.bfloat16)

    in_maps = []
    for k in range(NCORES):
        in_maps.append({
            "main": mo[BL * k:BL * (k + 1)].reshape(BL * C, HW),
            "dup": du[BL * k:BL * (k + 1)].reshape(BL * K, HW),
            "bounds": bounds,
            "ident": ident,
        })

    nc = _get_nc(HW)
    res = run_bass_kernel_spmd(nc, in_maps, list(range(NCORES)), **spmd_kwargs)
    out = np.concatenate(
        [r["out"].reshape(BL, C, H, W) for r in res.results], axis=0)

    if perm is not None:
        inv = np.empty(C, dtype=np.int64)
        inv[perm] = np.arange(C)
        out = out[:, inv]
    return out, res


# revision 28
# speedup vs baseline: 1.0383x; 1.0323x over previous
"""EDAC layer kernel for Trainium2 (8 NeuronCores, batch-sharded SPMD).

Reference semantics (B=32, C=256, K=64, H=W=56; vulnerable_idx == arange(K)):
  valid(x, c)  = min_vals[c] <= x <= max_vals[c]
  channels >= K:  out = x if valid else 0
  channels <  K:  m = main, d = dup
      both valid  -> min(m, d)      (covers m == d too)
      only d      -> d
      only m      -> m
      neither     -> 0

Kernel strategy (per core, 4 batches):
  rows = (batch, channel) pairs on SBUF partitions, H*W on the free dim.
  Per batch-pair (b, b+1) process five [128, HW] tiles:
    A: batch b   channels  64..191   (simple range-zero path)
    B: batch b   channels 192..255 + batch b+1 channels 64..127
    C: batch b+1 channels 128..255
    V: channels 0..63 of both batches (vulnerable, compared against dup)
    D: dup rows for both batches
  Simple path: two in-place scalar_tensor_tensor ops on VectorE
               ((m>=lo)*m, then (m<=hi)*that -- safe because 0 <= hi).
  Vulnerable:  ScalarE relus r1=relu(lo-x), r2=relu(x-hi) in bf16 (zero vs
               positive is exact), w = r1+r2 via TensorE identity-matmul
               accumulation into PSUM, sentinel m1 = m + HUGE*w on VectorE,
               r = min(m1, d1), res = (r < THR) * r.
  Engine/DMA plan: loads on the sync HWDGE ring (single FIFO = lowest
  first-tile latency), early stores via GPSIMD SWDGE, late stores on the
  then-idle sync ring.  B/V/D tiles interleave their two 64-row segments
  into even/odd partitions via [64, 2, hw] APs so every DMA keeps full
  128-partition port coverage across all 16 SDMA engines.
"""

import os
import sys

for _p in ("/opt/trn_rl_repo", os.path.expanduser("~/.axon_site/_ro/trn_rl_repo")):
    if os.path.isdir(_p) and _p not in sys.path:
        sys.path.insert(0, _p)

import numpy as np

import concourse.bass as bass
import concourse.bacc as bacc
import concourse.mybir as mybir
from concourse.tile import TileContext
from concourse.bass_utils import run_bass_kernel_spmd

F32 = mybir.dt.float32
BF16 = mybir.dt.bfloat16
OP = mybir.AluOpType
AF = mybir.ActivationFunctionType

B, C, K, H, W = 32, 256, 64, 56, 56
HW = H * W
NCORES = 8
BL = B // NCORES  # batches per core

HUGE = 1.0e30  # sentinel multiplier: HUGE * smallest-positive-bf16-relu >> THR
THR = 1.0e15   # valid values are <= ~10; invalid sentinels are >= ~6e22

# bounds table columns (per-partition scalars for each tile kind)
#   0..3  : lo  for tile kinds A, B, C, V
#   4..7  : hi  for tile kinds A, B, C, V
#   8..11 : -hi for tile kinds A, B, C, V
NBCOLS = 12


def build_bounds(min_vals: np.ndarray, max_vals: np.ndarray) -> np.ndarray:
    lo = np.asarray(min_vals, dtype=np.float32)
    hi = np.asarray(max_vals, dtype=np.float32)
    cols = np.zeros((128, NBCOLS), dtype=np.float32)
    interleave = lambda a, b: np.stack([a, b], axis=1).ravel()
    kinds = [
        np.arange(64, 192),                                   # A: ch 64..191
        interleave(np.arange(192, 256), np.arange(64, 128)),  # B (interleaved)
        np.arange(128, 256),                                  # C: ch 128..255
        np.repeat(np.arange(0, 64), 2),                       # V (interleaved)
    ]
    for j, idx in enumerate(kinds):
        cols[:, j] = lo[idx]
        cols[:, 4 + j] = hi[idx]
        cols[:, 8 + j] = -hi[idx]
    return cols


def build_nc(hw: int = HW) -> bass.Bass:
    nc = bacc.Bacc("TRN2", target_bir_lowering=False, debug=False)
    R = BL * C
    main = nc.dram_tensor("main", [R, hw], F32, kind="ExternalInput")
    dup = nc.dram_tensor("dup", [BL * K, hw], F32, kind="ExternalInput")
    bounds = nc.dram_tensor("bounds", [128, NBCOLS], F32, kind="ExternalInput")
    ident = nc.dram_tensor("ident", [128, 128], BF16, kind="ExternalInput")
    out = nc.dram_tensor("out", [R, hw], F32, kind="ExternalOutput")

    stt = nc.vector.scalar_tensor_tensor
    npairs = BL // 2

    # Per-pair DRAM views. B and V tiles interleave their two 64-row segments
    # into even/odd SBUF partitions via a [64, 2, hw] AP (outer dim 64), so a
    # single dma_start still spreads over all 16 SDMA engines with full
    # 128-partition port coverage (64-partition DMAs run at half BW; multi-
    # segment outer-dim-2 APs collapse onto 2 engines).
    main_p = main.ap().rearrange("(p x) w -> p x w", p=npairs)   # [p, 512, hw]
    out_p = out.ap().rearrange("(p x) w -> p x w", p=npairs)
    dup_p = dup.ap().rearrange("(p s c) w -> p c s w", p=npairs, s=2)

    def v_ap(t):   # [64, 2, hw]: ch 0..63 of batches b, b+1 interleaved
        return t.rearrange("(s g c) w -> g c s w", s=2, g=4)[0]

    def b_ap(t):   # [64, 2, hw]: ch 192..255 of b / ch 64..127 of b+1
        return t[192:384].rearrange("(s c) w -> c s w", s=3)[:, 0:3:2]

    APS = {
        0: lambda t: t[64:192],      # A
        1: b_ap,                     # B
        2: lambda t: t[384:512],     # C
    }

    with TileContext(nc) as tc:
        with (
            tc.tile_pool(name="bnd", bufs=1) as bpool,
            tc.tile_pool(name="pm", bufs=6) as pm,
            tc.tile_pool(name="pv", bufs=2) as pv,
            tc.tile_pool(name="pd", bufs=2) as pd,
            tc.tile_pool(name="pr", bufs=8) as pr,
            tc.tile_pool(name="pp", bufs=2, space="PSUM") as pp,
        ):
            bt = bpool.tile([128, NBCOLS], F32)
            nc.sync.dma_start(out=bt[:], in_=bounds[:])
            it = bpool.tile([128, 128], BF16, tag="ident")
            nc.sync.dma_start(out=it[:], in_=ident[:])

            def lo_ap(j):
                return bt[:, j:j + 1]

            def hi_ap(j):
                return bt[:, 4 + j:5 + j]

            def nhi_ap(j):
                return bt[:, 8 + j:9 + j]

            # Load-trigger order (scalar HWDGE ring) is tuned so the DVE
            # starts on A0 at ~13us while V/D of each pair still land early
            # enough to hide the ScalarE relu chain behind simple-tile DVE
            # work.  Tiles land ~4.4us apart while the ring streams.
            vd = [None] * npairs
            abc = [[None] * 3 for _ in range(npairs)]

            def load_vd(p):
                mv = pv.tile([128, hw], F32, tag="mv")
                nc.sync.dma_start(out=mv[:], in_=v_ap(main_p[p]))
                dv = pd.tile([128, hw], F32, tag="dv")
                nc.sync.dma_start(out=dv[:], in_=dup_p[p])
                vd[p] = (mv, dv)

            def load_simple(p, kind, split=False):
                mt = pm.tile([128, hw], F32, tag="mt")
                src_ap = APS[kind](main_p[p])
                if split:  # two half DMAs: first data lands sooner
                    h = hw // 2
                    nc.sync.dma_start(out=mt[:, 0:h], in_=src_ap[..., 0:h])
                    nc.sync.dma_start(out=mt[:, h:hw], in_=src_ap[..., h:hw])
                else:
                    nc.sync.dma_start(out=mt[:], in_=src_ap)
                abc[p][kind] = mt

            load_simple(0, 0, split=True)
            load_vd(0)
            load_simple(0, 1)
            load_vd(1)
            load_simple(0, 2)
            load_simple(1, 0)
            load_simple(1, 1)
            load_simple(1, 2)

            # ScalarE relu stream: vuln pairs first, then the two simple
            # tiles that take the relu+PE path (A1, B1).
            relus = []
            for p in range(npairs):
                mv, dv = vd[p]
                r1m = pr.tile([128, hw], BF16, tag="rl")
                r2m = pr.tile([128, hw], BF16, tag="rl")
                r1d = pr.tile([128, hw], BF16, tag="rl")
                r2d = pr.tile([128, hw], BF16, tag="rl")
                nc.scalar.activation(r1m[:], mv[:], AF.Relu, bias=lo_ap(3), scale=-1.0)
                nc.scalar.activation(r2m[:], mv[:], AF.Relu, bias=nhi_ap(3), scale=1.0)
                nc.scalar.activation(r1d[:], dv[:], AF.Relu, bias=lo_ap(3), scale=-1.0)
                nc.scalar.activation(r2d[:], dv[:], AF.Relu, bias=nhi_ap(3), scale=1.0)
                relus.append((r1m, r2m, r1d, r2d))
            half = hw // 2

            def pe_w(r1, r2, cs):
                """w = r1 + r2 on TensorE (identity matmuls into PSUM)."""
                w = pp.tile([128, half], F32, tag="w")
                for c0 in range(0, half, 512):
                    c1 = min(c0 + 512, half)
                    nc.tensor.matmul(w[:, c0:c1], it[:], r1[:, cs][:, c0:c1],
                                     start=True, stop=False)
                    nc.tensor.matmul(w[:, c0:c1], it[:], r2[:, cs][:, c0:c1],
                                     start=False, stop=True)
                return w

            def do_simple(p, kind, late=False, split=False):
                mt = abc[p][kind]
                eng = nc.sync if late else nc.gpsimd
                dst = APS[kind](out_p[p])
                halves = ((slice(0, half), slice(half, hw))
                          if split else (slice(0, hw),))
                for cs in halves:
                    stt(out=mt[:, cs], in0=mt[:, cs], scalar=lo_ap(kind),
                        in1=mt[:, cs], op0=OP.is_ge, op1=OP.mult)
                    stt(out=mt[:, cs], in0=mt[:, cs], scalar=hi_ap(kind),
                        in1=mt[:, cs], op0=OP.is_le, op1=OP.mult)
                    eng.dma_start(out=dst[..., cs], in_=mt[:, cs])

            def do_vuln(p):
                mv, dv = vd[p]
                r1m, r2m, r1d, r2d = relus[p]
                for src_pair, dst in (((r1m, r2m), mv), ((r1d, r2d), dv)):
                    for h in range(2):
                        cs = slice(h * half, (h + 1) * half)
                        w = pe_w(src_pair[0], src_pair[1], cs)
                        stt(out=dst[:, cs], in0=w[:], scalar=HUGE,
                            in1=dst[:, cs], op0=OP.mult, op1=OP.add)
                eng = nc.sync if p == npairs - 1 else nc.gpsimd
                vdst = v_ap(out_p[p])
                for h in range(2):
                    cs = slice(h * half, (h + 1) * half)
                    nc.vector.tensor_tensor(out=mv[:, cs], in0=mv[:, cs],
                                            in1=dv[:, cs], op=OP.min)
                    stt(out=dv[:, cs], in0=mv[:, cs], scalar=THR,
                        in1=mv[:, cs], op0=OP.is_lt, op1=OP.mult)
                    eng.dma_start(out=vdst[..., cs], in_=dv[:, cs])

            do_simple(0, 0, split=True)
            do_simple(0, 1)
            do_vuln(0)
            do_simple(0, 2)
            do_simple(1, 0)
            do_vuln(1)
            do_simple(1, 1, late=True)
            do_simple(1, 2, late=True, split=True)
    return nc


_NC_CACHE: dict = {}


def _get_nc(hw: int) -> bass.Bass:
    if hw not in _NC_CACHE:
        nc = build_nc(hw)
        nc.finalize()  # Bacc.finalize runs compile() (register allocation etc.)
        _NC_CACHE[hw] = nc
    return _NC_CACHE[hw]


def kernel(main_out, dup_out, min_vals, max_vals, vulnerable_idx):
    return _run(main_out, dup_out, min_vals, max_vals, vulnerable_idx)[0]


def _run(main_out, dup_out, min_vals, max_vals, vulnerable_idx, **spmd_kwargs):
    main_out = np.asarray(main_out)
    dup_out = np.asarray(dup_out)
    min_vals = np.asarray(min_vals)
    max_vals = np.asarray(max_vals)
    vidx = np.asarray(vulnerable_idx).ravel()

    # Device kernel assumes vulnerable channels are 0..K-1. If not, permute
    # channels host-side so they are, and invert on the way out.
    perm = None
    if not np.array_equal(vidx, np.arange(K)):
        assert len(np.unique(vidx)) == K, "duplicate vulnerable_idx unsupported"
        rest = np.setdiff1d(np.arange(C), vidx)
        perm = np.concatenate([vidx, rest])
        main_out = main_out[:, perm]
        min_vals = min_vals[perm]
        max_vals = max_vals[perm]

    mo = np.ascontiguousarray(main_out, dtype=np.float32).reshape(B, C, HW)
    du = np.ascontiguousarray(dup_out, dtype=np.float32).reshape(B, K, HW)
    bounds = build_bounds(min_vals, max_vals)
    import ml_dtypes
    ident = np.eye(128, dtype=ml_dtypes.bfloat16)

    in_maps = []
    for k in range(NCORES):
        in_maps.append({
            "main": mo[BL * k:BL * (k + 1)].reshape(BL * C, HW),
            "dup": du[BL * k:BL * (k + 1)].reshape(BL * K, HW),
            "bounds": bounds,
            "ident": ident,
        })

    nc = _get_nc(HW)
    res = run_bass_kernel_spmd(nc, in_maps, list(range(NCORES)), **spmd_kwargs)
    out = np.concatenate(
        [r["out"].reshape(BL, C, H, W) for r in res.results], axis=0)

    if perm is not None:
        inv = np.empty(C, dtype=np.int64)
        inv[perm] = np.arange(C)
        out = out[:, inv]
    return out, res
